# revision 8
# baseline (speedup 1.0000x reference)
"""DialogueRNN cell Bass kernel for Trainium2, 8-core data-parallel.

kernel(**inputs) takes the FULL inputs (as produced by
reference.setup_inputs()) and returns the full (global_state,
speaker_state, emotion) tuple. Internally it shards batch B=512 across
8 NeuronCores (64 rows each), uploads pre-transposed weights plus 0/1
selector layouts derived from the one-hot `speaker` input, runs one
SPMD Bass/Tile kernel, and concatenates the shards.

Device-side sketch (per core, 64-sample shard, all f32):
  * attention scores via fused DVE tensor_tensor_reduce over history
    tiles packed 4 timesteps x 32 samples per 128-partition tile
  * softmax with small cross-partition-group selector matmuls on PE
  * context via fused DVE scalar_tensor_tensor accumulation
  * all GRU gate GEMMs on PE with host-pre-transposed weights, biases
    folded in as K=1 matmuls; party/listener share one per-sample
    x-side GEMM which is expanded to the 8 party slots by a selector
    matmul; every gather/scatter over party slots is a
    speaker-weighted selector matmul.
"""

import sys as _sys
import types as _types

import numpy as np

B, T = 512, 128
F = G = P = E = 512
NP = 8
NCORES = 8
BS = B // NCORES          # 64 batch rows per core
BH = BS // 2              # 32-row half-shard for the attention passes
H = 512
H3 = 3 * H
DT = np.float32

_CACHE = {}

# ------------------------------------------------------------------
# post-pass: the walrus build in this image accepts at most one sync
# wait per TPB CTRL instruction and none on Drain; hoist excess waits
# onto preceding NoOps.
# ------------------------------------------------------------------
_wf = _types.ModuleType("waitfix_embedded")
exec(
    '''
import concourse.mybir as mybir
MAXW = 1
def split_sync_waits(nc):
    fixn = 0
    for f in nc.m.functions:
        for b in f.blocks:
            insts = list(b.instructions)
            out = []
            changed = False
            for ins in insts:
                si = ins.sync_info
                nw = len(si.on_wait) if si is not None else 0
                limit = 0 if ins.opcode == "Drain" else MAXW
                if si is not None and nw > limit:
                    waits = list(si.on_wait)
                    if limit:
                        extra, keep = waits[:-limit], waits[-limit:]
                    else:
                        extra, keep = waits, []
                    for i in range(0, len(extra), MAXW):
                        out.append(mybir.InstNoOp(
                            name=f"wf_{ins.name}_{i}",
                            engine=ins.engine,
                            bass_nofuse=True,
                            sync_info=mybir.SyncInfo(
                                on_wait=extra[i:i+MAXW], on_update=[])))
                        fixn += 1
                    ins.sync_info = mybir.SyncInfo(
                        on_wait=keep, on_update=list(si.on_update))
                    changed = True
                out.append(ins)
            if changed:
                b.instructions = out
    return fixn
''',
    _wf.__dict__,
)
_sys.modules["waitfix_embedded"] = _wf


# ------------------------------------------------------------------
# host-side prep
# ------------------------------------------------------------------

def _prep_gru(W_ih, W_hh, b_ih, b_hh):
    """Pre-transposed weights; biases packed as [1, 2048]:
    [ (b_ih+b_hh)[0:1024] | b_ih[1024:1536] | b_hh[1024:1536] ]."""
    WxT = np.ascontiguousarray(W_ih.T, dtype=DT)
    WhT = np.ascontiguousarray(W_hh.T, dtype=DT)
    bias = np.zeros((1, 2048), DT)
    bias[0, :1024] = b_ih[:1024] + b_hh[:1024]
    bias[0, 1024:1536] = b_ih[1024:]
    bias[0, 1536:] = b_hh[1024:]
    return WxT, WhT, bias


def _selectors():
    # E4g[h] [64,128]: replicate global rows h*32..h*32+31 into 4 groups.
    E4g = np.zeros((2, BS, 128), DT)
    for h in range(2):
        for j in range(128):
            E4g[h, h * BH + (j % BH), j] = 1.0
    # S4g[h] [128,64]: sum 4 groups back into global rows (transpose).
    S4g = np.ascontiguousarray(np.transpose(E4g, (0, 2, 1)))
    # E4l [64,128]: replicate local rows 0..31 into 4 groups.
    E4l = np.zeros((BS, 128), DT)
    for j in range(128):
        E4l[j % BH, j] = 1.0
    # S4l [128,64]: sum 4 groups into local rows 0..31.
    S4l = np.ascontiguousarray(E4l.T)
    # Esel [64,512]: expand per-sample row b to the 8 rows b*8+p.
    Esel = np.zeros((BS, BS * NP), DT)
    for bp in range(BS * NP):
        Esel[bp // NP, bp] = 1.0
    return E4g, S4g, E4l, S4l, Esel


# ------------------------------------------------------------------
# device program
# ------------------------------------------------------------------

def _build_program():
    import concourse.bass as bass
    import concourse.mybir as mybir
    import concourse.tile as tile
    from waitfix_embedded import split_sync_waits

    fp32 = mybir.dt.float32
    nc = bass.Bass("TRN2", target_bir_lowering=False, debug=False,
                   num_devices=NCORES)

    d = {}
    def din(name, shape):
        d[name] = nc.dram_tensor(name, list(shape), fp32,
                                 kind="ExternalInput")

    din("feat", (BS, F))
    din("lgs", (T, BS, G))
    din("lssf", (BS * NP, P))
    din("lem", (BS, E))
    din("e4g", (2, BS, 128))
    din("s4g", (2, 128, BS))
    din("e4l", (BS, 128))
    din("s4l", (128, BS))
    din("esel", (BS, BS * NP))
    din("gsel", (BS * NP, BS))
    din("maskbp", (4, 128))
    din("aWT", (F, G))
    for k in "gple":
        kin = 1024 if k in "gpl" else 512
        din(k + "WxT", (kin, H3))
        din(k + "WhT", (H, H3))
        din(k + "B", (1, 2048))

    og = nc.dram_tensor("og", [BS, G], fp32, kind="ExternalOutput")
    osp = nc.dram_tensor("osp", [BS * NP, P], fp32, kind="ExternalOutput")
    oe = nc.dram_tensor("oe", [BS, E], fp32, kind="ExternalOutput")

    with tile.TileContext(nc) as tc:
        _emit(nc, tc, d, og, osp, oe)

    split_sync_waits(nc)
    return nc


def _emit(nc, tc, d, og, osp, oe):
    from contextlib import ExitStack

    import concourse.mybir as mybir
    from concourse.masks import make_identity

    fp32 = mybir.dt.float32
    AF = mybir.ActivationFunctionType
    OP = mybir.AluOpType
    AX = mybir.AxisListType

    ctx = ExitStack()
    pool = lambda name, bufs, **kw: ctx.enter_context(
        tc.tile_pool(name=name, bufs=bufs, **kw))

    const = pool("const", 1)
    acts = pool("acts", 1)
    wx = pool("wx", 2)
    wh = pool("wh", 4)
    evict = pool("evict", 5)
    lgsp = pool("lgsp", 1)
    pg = pool("pg", 1, space="PSUM")    # 4 gate banks (tags A-D)
    pq = pool("pq", 1, space="PSUM")    # 4 banks (tags below)

    MM = nc.tensor.matmul
    ACT = nc.scalar.activation
    V = nc.vector

    # ---------------- constants ----------------
    ident = const.tile([128, 128], fp32)
    make_identity(nc, ident[:])
    ones = const.tile([1, 128], fp32)
    V.memset(ones[:], 1.0)
    e4g_sb = const.tile([64, 2, 128], fp32)
    s4g_sb = const.tile([128, 2, 64], fp32)
    for h in range(2):
        nc.sync.dma_start(out=e4g_sb[:, h, :], in_=d["e4g"].ap()[h])
        nc.sync.dma_start(out=s4g_sb[:, h, :], in_=d["s4g"].ap()[h])
    e4l_sb = const.tile([64, 128], fp32)
    nc.sync.dma_start(out=e4l_sb[:], in_=d["e4l"].ap())
    s4l_sb = const.tile([128, 64], fp32)
    nc.sync.dma_start(out=s4l_sb[:], in_=d["s4l"].ap())
    esel_sb = const.tile([64, 512], fp32)
    nc.sync.dma_start(out=esel_sb[:], in_=d["esel"].ap())
    gsel_sb = const.tile([128, 4, 64], fp32)
    for m in range(4):
        nc.sync.dma_start(out=gsel_sb[:, m, :],
                          in_=d["gsel"].ap()[m * 128:(m + 1) * 128, :])
    mask_sb = const.tile([128, 4], fp32)
    nc.sync.dma_start(out=mask_sb[:],
                      in_=d["maskbp"].ap().rearrange("m p -> p m"))
    def load_bias(key):
        bs = acts.tile([1, 2048], fp32, tag="bias", name=f"bias_{key}")
        nc.sync.dma_start(out=bs[:], in_=d[key + "B"].ap())
        return bs
    aWT_sb = const.tile([128, 4, 512], fp32)
    for c in range(4):
        nc.sync.dma_start(out=aWT_sb[:, c, :],
                          in_=d["aWT"].ap()[c * 128:(c + 1) * 128, :])

    # ---------------- small activations ----------------
    feat_sb = acts.tile([64, 512], fp32)
    nc.sync.dma_start(out=feat_sb[:], in_=d["feat"].ap())
    lem_sb = acts.tile([64, 512], fp32)
    nc.sync.dma_start(out=lem_sb[:], in_=d["lem"].ap())
    hg_sb = acts.tile([64, 512], fp32)
    nc.sync.dma_start(out=hg_sb[:], in_=d["lgs"].ap()[T - 1])
    lssf_sb = acts.tile([128, 4, 512], fp32)
    for m in range(4):
        nc.sync.dma_start(out=lssf_sb[:, m, :],
                          in_=d["lssf"].ap()[m * 128:(m + 1) * 128, :])

    def transpose_64(src_ap, dstT, c):
        pt = pq.tile([128, 512], fp32, tag="ptr")
        nc.tensor.transpose(pt[:, 0:64], src_ap, ident[0:64, 0:64])
        ACT(dstT[:, c, :], pt[:, 0:64], AF.Copy)

    def transpose_128(src_ap, dst_ap):
        pt = pq.tile([128, 512], fp32, tag="ptr")
        nc.tensor.transpose(pt[:, 0:128], src_ap, ident[:])
        ACT(dst_ap, pt[:, 0:128], AF.Copy)

    xTf = acts.tile([128, 4, 64], fp32)
    for c in range(4):
        transpose_64(feat_sb[:, c * 128:(c + 1) * 128], xTf, c)
    hgT = acts.tile([128, 4, 64], fp32)
    for c in range(4):
        transpose_64(hg_sb[:, c * 128:(c + 1) * 128], hgT, c)
    lemT = acts.tile([128, 4, 64], fp32)
    for c in range(4):
        transpose_64(lem_sb[:, c * 128:(c + 1) * 128], lemT, c)
    lssT = acts.tile([128, 4, 512], fp32)       # [h-chunk, kc, bp]
    for kc in range(4):
        for m in range(4):
            transpose_128(lssf_sb[:, m, kc * 128:(kc + 1) * 128],
                          lssT[:, kc, m * 128:(m + 1) * 128])

    # sel_last = lss[b, spk_idx[b]] (speaker-weighted gather matmul)
    psel = pq.tile([64, 512], fp32, tag="sel")
    for m in range(4):
        MM(psel[:], gsel_sb[:, m, :], lssf_sb[:, m, :],
           start=(m == 0), stop=(m == 3))
    selL_sb = acts.tile([64, 512], fp32)
    ACT(selL_sb[:], psel[:], AF.Copy)
    selLT = acts.tile([128, 4, 64], fp32)
    for c in range(4):
        transpose_64(selL_sb[:, c * 128:(c + 1) * 128], selLT, c)

    # ---------------- attention ----------------
    # q = feat @ attn_W.T
    pqt = pq.tile([64, 512], fp32, tag="sel")
    for c in range(4):
        MM(pqt[:], xTf[:, c, :], aWT_sb[:, c, :],
           start=(c == 0), stop=(c == 3))
    q_sb = acts.tile([64, 512], fp32)
    ACT(q_sb[:], pqt[:], AF.Copy)

    ctx_ps = pq.tile([64, 512], fp32, tag="ctx")
    NT = T // 4
    scratch = acts.tile([128, 512], fp32)
    scratch2 = acts.tile([128, 512], fp32)
    ctxa = acts.tile([128, 512], fp32)
    lgs_sb = lgsp.tile([128, NT, 512], fp32)

    for half in range(2):
        b0 = half * BH
        # q replicated into the 4 row groups of this half
        p4 = pq.tile([128, 512], fp32, tag="ptr")
        MM(p4[:], e4g_sb[:, half, :], q_sb[:], start=True, stop=True)
        q4_sb = acts.tile([128, 512], fp32, tag="q4")
        ACT(q4_sb[:], p4[:], AF.Copy)

        scores = acts.tile([128, NT], fp32, tag="scores")
        for tp in range(NT):
            for g4 in range(4):
                nc.sync.dma_start(
                    out=lgs_sb[g4 * BH:(g4 + 1) * BH, tp, :],
                    in_=d["lgs"].ap()[4 * tp + g4, b0:b0 + BH, :])
            V.tensor_mul(scratch[:], q4_sb[:], lgs_sb[:, tp, :])
            ACT(scratch2[:], scratch[:], AF.Copy,
                accum_out=scores[:, tp:tp + 1])

        # softmax over all T per sample (rows live in 4 partition groups)
        rmax = acts.tile([128, 1], fp32, tag="rmax")
        V.tensor_reduce(rmax[:], scores[:], axis=AX.X, op=OP.max)
        mx = acts.tile([64, 4], fp32, tag="mx")
        for g2 in range(1, 4):
            nc.sync.dma_start(out=mx[0:BH, g2:g2 + 1],
                              in_=rmax[g2 * BH:(g2 + 1) * BH, :])
        bound = acts.tile([64, 1], fp32, tag="bound")
        V.tensor_max(bound[0:BH, :], rmax[0:BH, :], mx[0:BH, 1:2])
        V.tensor_max(bound[0:BH, :], bound[0:BH, :], mx[0:BH, 2:3])
        V.tensor_max(bound[0:BH, :], bound[0:BH, :], mx[0:BH, 3:4])
        negb = acts.tile([64, 1], fp32, tag="negb")
        V.memset(negb[:], 0.0)
        V.tensor_scalar_mul(negb[0:BH, :], bound[0:BH, :], -1.0)
        pnb = pq.tile([128, 512], fp32, tag="ptr")
        MM(pnb[:, 0:1], e4l_sb[:], negb[:], start=True, stop=True)
        negb4 = acts.tile([128, 1], fp32, tag="negb4")
        ACT(negb4[:], pnb[:, 0:1], AF.Copy)

        expp = acts.tile([128, NT], fp32, tag="expp")
        rsum = acts.tile([128, 1], fp32, tag="rsum")
        ACT(expp[:], scores[:], AF.Exp, bias=negb4[:], accum_out=rsum[:])
        psum1 = pq.tile([128, 512], fp32, tag="ptr")
        MM(psum1[0:64, 0:1], s4l_sb[:], rsum[:], start=True, stop=True)
        rs = acts.tile([64, 1], fp32, tag="rs")
        V.memset(rs[:], 0.0)
        V.reciprocal(rs[0:BH, :], psum1[0:BH, 0:1])
        prs = pq.tile([128, 512], fp32, tag="ptr")
        MM(prs[:, 0:1], e4l_sb[:], rs[:], start=True, stop=True)
        rs4 = acts.tile([128, 1], fp32, tag="rs4")
        ACT(rs4[:], prs[:, 0:1], AF.Copy)
        alpha = acts.tile([128, NT], fp32, tag="alpha")
        V.tensor_scalar_mul(alpha[:], expp[:], rs4[:])

        # context accumulation in the packed layout
        V.memset(ctxa[:], 0.0)
        for tp in range(NT):
            V.scalar_tensor_tensor(
                out=ctxa[:], in0=lgs_sb[:, tp, :],
                scalar=alpha[:, tp:tp + 1], in1=ctxa[:],
                op0=OP.mult, op1=OP.add)
        MM(ctx_ps[:], s4g_sb[:, half, :], ctxa[:],
           start=(half == 0), stop=(half == 1))

    ctx_sb = acts.tile([64, 512], fp32, tag="stage64")
    ACT(ctx_sb[:], ctx_ps[:], AF.Copy)
    ctxT = acts.tile([128, 4, 64], fp32)
    for c in range(4):
        transpose_64(ctx_sb[:, c * 128:(c + 1) * 128], ctxT, c)

    # ---------------- GRU machinery ----------------
    def load_w(name, kin, which):
        tiles = []
        p_, tag = (wx, "wx") if which == "x" else (wh, "wh")
        for kc in range(kin // 128):
            wt = p_.tile([128, H3], fp32, tag=tag)
            nc.sync.dma_start(out=wt[:],
                              in_=d[name].ap()[kc * 128:(kc + 1) * 128, :])
            tiles.append(wt)
        return tiles

    def gru_ew(M, a_r, a_z, i_n, h_n, h_nat_ap, out_ap):
        """out = n + z*(h-n), n = tanh(i_n + r*h_n), r/z = sigmoid."""
        r_sb = evict.tile([128, 512], fp32, tag="ev")
        z_sb = evict.tile([128, 512], fp32, tag="ev")
        ACT(r_sb[0:M, :], a_r[0:M, :], AF.Sigmoid)
        ACT(z_sb[0:M, :], a_z[0:M, :], AF.Sigmoid)
        t1 = evict.tile([128, 512], fp32, tag="ev")
        V.tensor_mul(t1[0:M, :], r_sb[0:M, :], h_n[0:M, :])
        V.tensor_add(t1[0:M, :], t1[0:M, :], i_n[0:M, :])
        n_sb = evict.tile([128, 512], fp32, tag="ev")
        ACT(n_sb[0:M, :], t1[0:M, :], AF.Tanh)
        t3 = evict.tile([128, 512], fp32, tag="ev")
        V.tensor_sub(t3[0:M, :], h_nat_ap, n_sb[0:M, :])
        V.tensor_mul(t3[0:M, :], z_sb[0:M, :], t3[0:M, :])
        V.tensor_add(out_ap, t3[0:M, :], n_sb[0:M, :])

    def gru_small(key, xT_list, hT, h_nat, out_sb):
        wxt = load_w(key + "WxT", 128 * len(xT_list), "x")
        wht = load_w(key + "WhT", H, "h")
        bs = load_bias(key)
        a_r = pg.tile([64, 512], fp32, tag="pgA")
        a_z = pg.tile([64, 512], fp32, tag="pgB")
        i_n = pg.tile([64, 512], fp32, tag="pgC")
        h_n = pg.tile([64, 512], fp32, tag="pgD")
        for kc, xT in enumerate(xT_list):
            MM(a_r[:], xT, wxt[kc][:, 0:512], start=(kc == 0), stop=False)
            MM(a_z[:], xT, wxt[kc][:, 512:1024], start=(kc == 0), stop=False)
            MM(i_n[:], xT, wxt[kc][:, 1024:1536], start=(kc == 0), stop=False)
        for kc in range(4):
            MM(a_r[:], hT[:, kc, :], wht[kc][:, 0:512], start=False,
               stop=False)
            MM(a_z[:], hT[:, kc, :], wht[kc][:, 512:1024], start=False,
               stop=False)
            MM(h_n[:], hT[:, kc, :], wht[kc][:, 1024:1536], start=(kc == 0),
               stop=False)
        MM(a_r[:], ones[0:1, 0:64], bs[0:1, 0:512], start=False, stop=True)
        MM(a_z[:], ones[0:1, 0:64], bs[0:1, 512:1024], start=False, stop=True)
        MM(i_n[:], ones[0:1, 0:64], bs[0:1, 1024:1536], start=False,
           stop=True)
        MM(h_n[:], ones[0:1, 0:64], bs[0:1, 1536:2048], start=False,
           stop=True)
        gru_ew(64, a_r, a_z, i_n, h_n, h_nat[0:64, :], out_sb[0:64, :])

    def gru_party(key, xT_extra, out_tile):
        wxt = load_w(key + "WxT", 1024, "x")
        wht = load_w(key + "WhT", H, "h")
        bs = load_bias(key)
        # per-sample x-side gates (+ all biases except b_hh_n)
        gparts = []
        for j in range(3):
            ps = pg.tile([64, 512], fp32, tag="pg" + "ABC"[j])
            for kc in range(8):
                xT = xTf[:, kc, :] if kc < 4 else xT_extra[:, kc - 4, :]
                MM(ps[:], xT, wxt[kc][:, j * 512:(j + 1) * 512],
                   start=(kc == 0), stop=False)
            MM(ps[:], ones[0:1, 0:64], bs[0:1, j * 512:(j + 1) * 512],
               start=False, stop=True)
            gparts.append(ps)
        gi_sb = acts.tile([64, 3, 512], fp32, tag="gi")
        for j in range(3):
            ACT(gi_sb[:, j, :], gparts[j][:], AF.Copy)

        for m in range(4):
            em = esel_sb[:, m * 128:(m + 1) * 128]
            ms = slice(m * 128, (m + 1) * 128)
            a_r = pg.tile([128, 512], fp32, tag="pgA")
            a_z = pg.tile([128, 512], fp32, tag="pgB")
            i_n = pg.tile([128, 512], fp32, tag="pgC")
            h_n = pg.tile([128, 512], fp32, tag="pgD")
            MM(a_r[:], em, gi_sb[:, 0, :], start=True, stop=False)
            MM(a_z[:], em, gi_sb[:, 1, :], start=True, stop=False)
            MM(i_n[:], em, gi_sb[:, 2, :], start=True, stop=True)
            for kc in range(4):
                MM(a_r[:], lssT[:, kc, ms], wht[kc][:, 0:512], start=False,
                   stop=(kc == 3))
                MM(a_z[:], lssT[:, kc, ms], wht[kc][:, 512:1024],
                   start=False, stop=(kc == 3))
                MM(h_n[:], lssT[:, kc, ms], wht[kc][:, 1024:1536],
                   start=(kc == 0), stop=False)
            MM(h_n[:], ones[0:1, 0:128], bs[0:1, 1536:2048], start=False,
               stop=True)
            gru_ew(128, a_r, a_z, i_n, h_n, lssf_sb[:, m, :],
                   out_tile[:, m, :])

    # ---------------- global GRU ----------------
    og_sb = acts.tile([64, 512], fp32, tag="out64")
    gru_small("g",
              [xTf[:, c, :] for c in range(4)]
              + [selLT[:, c, :] for c in range(4)],
              hgT, hg_sb, og_sb)
    nc.sync.dma_start(out=og.ap(), in_=og_sb[0:64, :])

    # ---------------- party GRU ----------------
    ph = acts.tile([128, 4, 512], fp32)
    gru_party("p", ctxT, ph)

    # sel_spk gather
    pss = pq.tile([64, 512], fp32, tag="sel")
    for m in range(4):
        MM(pss[:], gsel_sb[:, m, :], ph[:, m, :],
           start=(m == 0), stop=(m == 3))
    ssp_sb = acts.tile([64, 512], fp32, tag="stage64")
    ACT(ssp_sb[:], pss[:], AF.Copy)
    sspT = acts.tile([128, 4, 64], fp32)
    for c in range(4):
        transpose_64(ssp_sb[:, c * 128:(c + 1) * 128], sspT, c)

    # ---------------- listener GRU ----------------
    lh = acts.tile([128, 4, 512], fp32)
    gru_party("l", sspT, lh)

    # ---------------- combine + speaker_state output ----------------
    for m in range(4):
        diff = evict.tile([128, 512], fp32, tag="ev")
        V.tensor_sub(diff[:], ph[:, m, :], lh[:, m, :])
        V.scalar_tensor_tensor(
            out=ph[:, m, :], in0=diff[:], scalar=mask_sb[:, m:m + 1],
            in1=lh[:, m, :], op0=OP.mult, op1=OP.add)
        nc.sync.dma_start(out=osp.ap()[m * 128:(m + 1) * 128, :],
                          in_=ph[:, m, :])

    # ---------------- emotion GRU ----------------
    pse = pq.tile([64, 512], fp32, tag="sel")
    for m in range(4):
        MM(pse[:], gsel_sb[:, m, :], ph[:, m, :],
           start=(m == 0), stop=(m == 3))
    sel_sb = acts.tile([64, 512], fp32, tag="stage64")
    ACT(sel_sb[:], pse[:], AF.Copy)
    selT2 = acts.tile([128, 4, 64], fp32)
    for c in range(4):
        transpose_64(sel_sb[:, c * 128:(c + 1) * 128], selT2, c)

    oe_sb = acts.tile([64, 512], fp32, tag="out64")
    gru_small("e", [selT2[:, c, :] for c in range(4)], lemT, lem_sb, oe_sb)
    nc.sync.dma_start(out=oe.ap(), in_=oe_sb[0:64, :])

    ctx.close()


# ------------------------------------------------------------------
# entry point
# ------------------------------------------------------------------

def kernel(**inputs):
    from concourse.bass_utils import run_bass_kernel_spmd

    if "nc" not in _CACHE:
        _CACHE["nc"] = _build_program()
    nc = _CACHE["nc"]

    f32 = lambda x: np.ascontiguousarray(np.asarray(x), dtype=DT)
    feature = f32(inputs["feature"])
    speaker = f32(inputs["speaker"])
    lgs = f32(inputs["last_global_state"])
    lss = f32(inputs["last_speaker_state"])
    lem = f32(inputs["last_emotion"])

    gW = _prep_gru(f32(inputs["gW_ih"]), f32(inputs["gW_hh"]),
                   f32(inputs["gb_ih"]), f32(inputs["gb_hh"]))
    pW = _prep_gru(f32(inputs["pW_ih"]), f32(inputs["pW_hh"]),
                   f32(inputs["pb_ih"]), f32(inputs["pb_hh"]))
    lW = _prep_gru(f32(inputs["lW_ih"]), f32(inputs["lW_hh"]),
                   f32(inputs["lb_ih"]), f32(inputs["lb_hh"]))
    eW = _prep_gru(f32(inputs["eW_ih"]), f32(inputs["eW_hh"]),
                   f32(inputs["eb_ih"]), f32(inputs["eb_hh"]))
    aWT = np.ascontiguousarray(f32(inputs["attn_W"]).T)

    E4g, S4g, E4l, S4l, Esel = _selectors()

    in_maps = []
    for c in range(NCORES):
        sl = slice(c * BS, (c + 1) * BS)
        spk = speaker[sl]
        gsel = np.zeros((BS * NP, BS), DT)
        for b in range(BS):
            gsel[b * NP:(b + 1) * NP, b] = spk[b]
        maskbp = np.ascontiguousarray(spk.reshape(-1).reshape(4, 128))
        in_maps.append(dict(
            feat=feature[sl],
            lgs=np.ascontiguousarray(lgs[:, sl, :]),
            lssf=np.ascontiguousarray(lss[sl].reshape(BS * NP, P)),
            lem=lem[sl],
            e4g=E4g, s4g=S4g, e4l=E4l, s4l=S4l, esel=Esel,
            gsel=gsel, maskbp=maskbp, aWT=aWT,
            gWxT=gW[0], gWhT=gW[1], gB=gW[2],
            pWxT=pW[0], pWhT=pW[1], pB=pW[2],
            lWxT=lW[0], lWhT=lW[1], lB=lW[2],
            eWxT=eW[0], eWhT=eW[1], eB=eW[2],
        ))

    _CACHE["in_maps"] = in_maps
    res = run_bass_kernel_spmd(nc, in_maps, list(range(NCORES)))
    gs = np.concatenate([res.results[c]["og"] for c in range(NCORES)], axis=0)
    sp = np.concatenate(
        [res.results[c]["osp"].reshape(BS, NP, P) for c in range(NCORES)],
        axis=0)
    em = np.concatenate([res.results[c]["oe"] for c in range(NCORES)], axis=0)
    return gs, sp, em
